# revision 31
# baseline (speedup 1.0000x reference)
# Trainium2 Bass kernel for nn_ComplexLambdaLayer (complex lambda attention layer).
# Sharding: data-parallel over batch b (16) across 8 cores (2 per core).
# The positional-lambda contraction lam_p[b,n,k,v] = sum_m R_k[n,m] V[b,v,m] uses
# the block-Toeplitz structure of R (R[n,m] = emb[pos_m - pos_n + 31]): only 15
# distinct 128x128 blocks per k exist (host-expanded fp16 table, d-major), so the
# 1024x1024 matmul becomes 8x8 chunk-matmuls with 15 stationary weights.
# lam_c is folded into the same PSUM chain via an indicator-row matmul.
# Yp = sum_k q*Lam uses a block-diagonal q lhsT (8 n-positions x 16 k = K128).
#
# Device schedule: nb-outer / k-inner main loop with a 9-slot rolling ring of mk
# d-blocks in SBUF; the BN AllReduce overlaps softmax/ksmT; the post-AR v-path is
# emitted first because it gates the matmuls.  TimelineSim ~291us.
#
# End-to-end wall time over the axon tunnel (~45 MB/s up, ~34 MB/s down, ~85 ms
# RTT) is dominated by host<->device transfer and per-call jit rebuild, so the
# runner here:
#   - builds the Bass module and the jitted shard_map callable ONCE (_CACHE);
#   - keeps all weight-derived constant tables (mk 15.7MB/core, wstk, eyerow,
#     ident, bnp) device-resident across calls, revalidated by byte-compare of
#     the small weight inputs;
#   - ships x as fp16 (16MB/call up) and returns yr/yi as fp16 (16MB/call down);
#   - does not upload donated zero output buffers (kernel writes every output
#     element).
import numpy as np
from contextlib import ExitStack

import bass_rust
import concourse.bacc as bacc
import concourse.tile as tile
from concourse import mybir

F32 = mybir.dt.float32
F16 = mybir.dt.float16
I8 = mybir.dt.int8

NCORES = 8
B = 16
BL = 2          # batches per core
DIM = 256
KD = 16         # DIM_K
HEADS = 8
VD = 32         # DIM_V
N2 = 1024
EPS = 1e-5
NSTAT = float(B * N2)

_CACHE = {}

# Y is stored as Y*_OSCALE in fp16 on device (power of two: exact rescale).
_OSCALE = 1.0 / 16.0

_CONST_KEYS = ('wq_re', 'wq_im', 'wk_re', 'wk_im', 'wv_re', 'wv_im',
               'qs_re', 'qs_im', 'qb_re', 'qb_im', 'vs_re', 'vs_im',
               'vb_re', 'vb_im', 'emb_re', 'emb_im')


def _build_host_consts(inp):
    # --- M_all: lhsT[(m-chunk),(n-chunk)] = R[n,m] = emb[pos_m - pos_n + 31]
    # M[k, dp+7][ap*32+jp, a*32+j] = emb[4dp + ap - a + 31, jp - j + 31, k, 0]
    er, ei = inp['emb_re'], inp['emb_im']
    a = np.arange(4); j = np.arange(32); dp = np.arange(-7, 8)
    r0 = (4 * dp[:, None, None, None, None] + a[None, :, None, None, None]
          - a[None, None, None, :, None] + 31)
    r1 = j[None, None, :, None, None] - j[None, None, None, None, :] + 31
    r0 = np.broadcast_to(r0, (15, 4, 32, 4, 32))
    r1 = np.broadcast_to(r1, (15, 4, 32, 4, 32))
    Mr = np.moveaxis(er[r0, r1, :, 0], -1, 0).reshape(16, 15, 128, 128)
    Mi = np.moveaxis(ei[r0, r1, :, 0], -1, 0).reshape(16, 15, 128, 128)
    # mk layout: d-major [d 15][p 128][(k,ri,c) 4096] fp16 (rolling-ring loads)
    # Scaled by OSCALE so Y (which can reach ~1e5 and overflow fp16) is stored
    # as Y*OSCALE in the fp16 outputs; host assembly multiplies back.
    mk = np.empty((15, 128, 16 * 2 * 128), np.float16)
    for k in range(16):
        mk[:, :, k * 256:k * 256 + 128] = Mr[k].transpose(0, 1, 2)
        mk[:, :, k * 256 + 128:k * 256 + 256] = Mi[k]
    mk *= np.float16(_OSCALE)

    # --- W stacks: proj out_r chain uses [Wr; -Wi], out_i chain [Wi; Wr]
    # o-order: 0-127 q(h,k), 128-143 k-proj, 144-175 v-proj; lhsT layout [c,o]
    # q channel order h*16+k (h-major, so qTg k-col slices are contiguous)
    Wr = np.concatenate([inp['wq_re'], inp['wk_re'], inp['wv_re']], 0).T
    Wi = np.concatenate([inp['wq_im'], inp['wk_im'], inp['wv_im']], 0).T
    # cols: 0-127 q, 128-143 k, 144-159 pad, 160-191 v (32-aligned psum bases)
    Wr = np.concatenate([Wr[:, :144], np.zeros((256, 16), np.float32), Wr[:, 144:]], 1)
    Wi = np.concatenate([Wi[:, :144], np.zeros((256, 16), np.float32), Wi[:, 144:]], 1)
    wstk = np.empty((2, 2, 2, 128, 192), np.float16)  # [outri][cc][ri_in]
    for cc in range(2):
        sl = slice(cc * 128, (cc + 1) * 128)
        wstk[0, cc, 0] = Wr[sl]; wstk[0, cc, 1] = -Wi[sl]
        wstk[1, cc, 0] = Wi[sl]; wstk[1, cc, 1] = Wr[sl]

    # --- eyerow for lam_c fold: [16, 16*128] fp16, eyerow[kk, k*128+c] = (kk==k)
    # (scaled by OSCALE like mk so lam_c and lam_p carry the same factor)
    eyerow = np.zeros((16, 16 * 128), np.float16)
    for k in range(16):
        eyerow[k, k * 128:(k + 1) * 128] = np.float16(_OSCALE)

    ident = np.eye(128, dtype=np.float32)
    ident16 = np.eye(128, dtype=np.float16)

    # --- BN params tile [128, 8]: q Ar-src cols 0-3 (qs_r qs_i qb_r qb_i),
    # v on rows 0-31 cols 4-7
    bnp = np.zeros((128, 8), np.float32)
    bnp[:, 0] = inp['qs_re']; bnp[:, 1] = inp['qs_im']
    bnp[:, 2] = inp['qb_re']; bnp[:, 3] = inp['qb_im']
    bnp[:32, 4] = inp['vs_re']; bnp[:32, 5] = inp['vs_im']
    bnp[:32, 6] = inp['vb_re']; bnp[:32, 7] = inp['vb_im']
    return {"wstk": wstk.reshape(8, 128, 192), "mk": mk, "eyerow": eyerow,
            "ident": ident, "ident16": ident16, "bnp": bnp}


def _build_nc():
    nc = bacc.Bacc("TRN2", target_bir_lowering=False, num_devices=NCORES)
    # x arrives fp16. (int8 x was tried and fails: the complex-BN variance is a
    # near-cancelling difference, so the layer amplifies input quantization
    # noise ~30-100x through 1/sqrt(var+eps); fp16 is the coarsest viable.)
    # xsc (col0 = 1/s, col1 = 1/s^2) descales the BN variance and softmax |k|^2
    # for a host-chosen input scale s; with fp16 x it is fed s = 1.
    xf_d = nc.declare_dram_parameter("xf", [8, 128, N2], F16, isOutput=False)
    xsc_d = nc.declare_dram_parameter("xsc", [128, 2], F32, isOutput=False)
    w_d = nc.declare_dram_parameter("wstk", [8, 128, 192], F16, isOutput=False)
    mk_d = nc.declare_dram_parameter("mk", [15, 128, 4096], F16, isOutput=False)
    eye_d = nc.declare_dram_parameter("eyerow", [16, 2048], F16, isOutput=False)
    id_d = nc.declare_dram_parameter("ident", [128, 128], F32, isOutput=False)
    id16_d = nc.declare_dram_parameter("ident16", [128, 128], F16, isOutput=False)
    bnp_d = nc.declare_dram_parameter("bnp", [128, 8], F32, isOutput=False)
    # Y is returned int8-quantized per partition row (yq) with the inverse
    # scales in ysc[p, b*2+ri]; the host dequantizes and assembles.
    yq_d = nc.declare_dram_parameter("yq", [BL, 2, 64, 4096], I8, isOutput=True)
    ysc_d = nc.declare_dram_parameter("ysc", [64, 4], F32, isOutput=True)
    arin = nc.dram_tensor("arin", [128, 10], F32)
    arout = nc.dram_tensor("arout", [128, 10], F32, addr_space="Shared")
    lamdram = nc.dram_tensor("lamdram", [2, 128, 2048], F16)
    qdram = nc.dram_tensor("qdram", [2, 128, 4096], F16)

    with tile.TileContext(nc) as tc, ExitStack() as ctx:
        per = ctx.enter_context(tc.tile_pool(name="per", bufs=1))   # persistent
        tmp = ctx.enter_context(tc.tile_pool(name="tmp", bufs=2))   # scratch
        tmp1 = ctx.enter_context(tc.tile_pool(name="tmp1", bufs=1))  # scratch, single

        wt = [per.tile([128, 192], F16, tag=f"w{i}", name=f"w{i}") for i in range(8)]
        eye = per.tile([16, 2048], F16, tag="eye", name="eye")
        nc.sync.dma_start(eye[:], eye_d[:])
        ident = per.tile([128, 128], F32, tag="ident", name="ident")
        nc.sync.dma_start(ident[:], id_d[:])
        ident16 = per.tile([128, 128], F16, tag="ident16", name="ident16")
        nc.sync.dma_start(ident16[:], id16_d[:])
        bnp = per.tile([128, 8], F32, tag="bnp", name="bnp")
        nc.sync.dma_start(bnp[:], bnp_d[:])
        xsc = per.tile([128, 2], F32, tag="xsc", name="xsc")
        nc.sync.dma_start(xsc[:], xsc_d[:])

        # rolling 9-slot mk ring: slot s holds d-block with d % 9 == s
        mkc = per.tile([128, 9 * 4096], F16, tag="mkc", name="mkc")

        q16 = [[per.tile([128, N2], F16, tag=f"q16{b}{ri}", name=f"q16{b}{ri}")
                for ri in range(2)] for b in range(BL)]
        k_sb = [[per.tile([16, N2], F16, tag=f"k{b}{ri}", name=f"k{b}{ri}")
                 for ri in range(2)] for b in range(BL)]
        v16 = [[per.tile([32, N2], F16, tag=f"v16{b}{ri}", name=f"v16{b}{ri}")
                for ri in range(2)] for b in range(BL)]

        # ---------------- projections (fp16, N=512) ----------------
        with tc.tile_pool(name="xfp", bufs=1) as xfp, \
             tc.tile_pool(name="pj", bufs=4, space="PSUM") as pj:
            xft = [xfp.tile([128, N2], F16, tag=f"xf{i % 6}", name=f"xf{i}") for i in range(8)]
            for i in range(8):
                nc.sync.dma_start(wt[i][:], w_d[i])
            for i in range(6):
                nc.sync.dma_start(xft[i][:], xf_d[i])
            for i in range(6, 8):
                nc.sync.dma_start(xft[i][:], xf_d[i])
            for d in range(7, 15):
                # 1-elem dep copy: delays the big mkc load until all xf arrived
                nc.vector.tensor_copy(mkc[0:1, (d % 9) * 4096:(d % 9) * 4096 + 1],
                                      xft[7][0:1, 0:1])
                nc.scalar.dma_start(mkc[:, (d % 9) * 4096:(d % 9 + 1) * 4096], mk_d[d])
            for b in range(BL):
                for ri in range(2):
                    for nch in range(2):
                        pq = pj.tile([128, 512], F32, tag="pq", name="pq")
                        pkv = pj.tile([64, 512], F32, tag="pkv", name="pkv")
                        first = True
                        for cc in range(2):
                            for rin in range(2):
                                lhs = wt[ri * 4 + cc * 2 + rin]
                                rhs = xft[b * 4 + rin * 2 + cc][:, nch * 512:(nch + 1) * 512]
                                nc.tensor.matmul(pq[:], lhs[:, 0:128], rhs,
                                                 start=first, stop=(cc == 1 and rin == 1))
                                nc.tensor.matmul(pkv[:], lhs[:, 128:192], rhs,
                                                 start=first, stop=(cc == 1 and rin == 1))
                                first = False
                        sl = slice(nch * 512, (nch + 1) * 512)
                        nc.scalar.copy(q16[b][ri][:, sl], pq[:])
                        nc.scalar.copy(k_sb[b][ri][:, sl], pkv[0:16, :])
                        nc.scalar.copy(v16[b][ri][:, sl], pkv[32:64, :])

        # ---------------- BN stats + AllReduce ----------------
        stats = per.tile([128, 10], F32, tag="stats", name="stats")
        nc.vector.memset(stats[:], 0.0)
        st_sc = [tmp.tile([128, 1], F32, tag=f"sc{i}", name=f"sc{i}") for i in range(4)]
        scr16 = [tmp1.tile([128, N2], F16, tag=f"s16{i}", name=f"s16{i}") for i in range(2)]

        statsP = per.tile([128, 20], F32, tag="statsP", name="statsP")

        def stat5_b(planes, rows, coff, b):
            # one batch's 5 partial stats -> statsP[:, coff + b*5 + s]
            pr, pi = planes[b][0][0:rows, :], planes[b][1][0:rows, :]
            for s_i, expr in enumerate(["r", "i", "rr", "ii", "ri"]):
                t = statsP[0:rows, coff + b * 5 + s_i:coff + b * 5 + s_i + 1]
                if expr == "r":
                    nc.vector.tensor_reduce(t, pr, mybir.AxisListType.X,
                                            mybir.AluOpType.add)
                elif expr == "i":
                    nc.vector.tensor_reduce(t, pi, mybir.AxisListType.X,
                                            mybir.AluOpType.add)
                else:
                    a_, b_ = (pr, pr) if expr == "rr" else (pi, pi) if expr == "ii" else (pr, pi)
                    nc.vector.tensor_mul(scr16[b][0:rows, :], a_, b_)
                    nc.vector.tensor_reduce(t, scr16[b][0:rows, :],
                                            mybir.AxisListType.X, mybir.AluOpType.add)

        def stat5(planes, rows, coff):
            for s_i in range(5):
                nc.vector.tensor_add(
                    stats[0:rows, coff + s_i:coff + s_i + 1],
                    statsP[0:rows, coff * 2 + s_i:coff * 2 + s_i + 1],
                    statsP[0:rows, coff * 2 + 5 + s_i:coff * 2 + 5 + s_i + 1])

        stat5_b(q16, 128, 0, 0)
        stat5_b(v16, 32, 10, 0)
        stat5_b(q16, 128, 0, 1)
        stat5_b(v16, 32, 10, 1)
        stat5(q16, 128, 0)
        stat5(v16, 32, 5)
        nc.sync.dma_start(arin[:], stats[:])
        nc.gpsimd.collective_compute(
            "AllReduce", mybir.AluOpType.add,
            replica_groups=[list(range(NCORES))],
            ins=[arin[:]], outs=[arout[:]])
        ar = per.tile([128, 10], F32, tag="ar", name="ar")
        nc.sync.dma_start(ar[:], arout[:])

        # ---------------- softmax(|k|) -> ksmT (overlaps AllReduce) ----------------
        ksmT = [per.tile([128, 128], F16, tag=f"ksmT{b}", name=f"ksmT{b}") for b in range(BL)]
        scrap = [tmp1.tile([16, N2], F32, tag=f"scr{i}", name=f"scr{i}") for i in range(2)]
        with tc.tile_pool(name="tp", bufs=2, space="PSUM") as tpp:
            for b in range(BL):
                kr, ki = k_sb[b][0], k_sb[b][1]
                ka = scrap[0][0:16, :]
                t1 = scrap[1][0:16, :]
                nc.vector.tensor_mul(ka, kr, kr)
                nc.vector.tensor_mul(t1, ki, ki)
                nc.vector.tensor_add(ka, ka, t1)
                # |k|^2 carries the s^2 input scaling; descale before sqrt
                nc.vector.tensor_scalar_mul(ka, ka, xsc[0:16, 1:2])
                nc.scalar.sqrt(ka, ka)
                mx = st_sc[2][0:16, :]
                nc.vector.tensor_reduce(mx, ka, mybir.AxisListType.X, mybir.AluOpType.max)
                nc.vector.tensor_scalar(ka, ka, mx, None, mybir.AluOpType.subtract)
                sm = st_sc[3][0:16, :]
                nc.scalar.activation(ka, ka, mybir.ActivationFunctionType.Exp,
                                     accum_out=sm)
                rc = st_sc[2][0:16, :]
                nc.vector.reciprocal(rc, sm)
                nc.vector.tensor_scalar(ka, ka, rc, None, mybir.AluOpType.mult)
                for ch in range(8):
                    pt = tpp.tile([128, 16], F32, tag="pt", name="pt")
                    nc.tensor.transpose(pt[:], ka[:, ch * 128:(ch + 1) * 128],
                                        ident[0:16, 0:16])
                    nc.vector.tensor_copy(ksmT[b][:, ch * 16:(ch + 1) * 16], pt[:])

        # ---------------- BN coefficients ----------------
        coef = per.tile([128, 8], F32, tag="coef", name="coef")   # q: Ar Ai Br Bi cols0-3; v cols4-7
        ct = [tmp.tile([128, 1], F32, tag=f"ct{i}", name=f"ct{i}") for i in range(8)]

        def bn_coef(rows, soff, poff, coff):
            r_ = slice(0, rows)
            mr, mi, t0, t1, t2, t3, sr, si = (c[r_, :] for c in ct)
            A = lambda c: ar[r_, soff + c:soff + c + 1]
            P = lambda c: bnp[r_, poff + c:poff + c + 1]
            C = lambda c: coef[r_, coff + c:coff + c + 1]
            inv = 1.0 / NSTAT
            nc.vector.tensor_scalar_mul(mr, A(0), inv)
            nc.vector.tensor_scalar_mul(mi, A(1), inv)
            # zr = (err - eii)/N - mr^2 + mi^2 + EPS
            nc.vector.tensor_sub(t0, A(2), A(3))
            nc.vector.tensor_scalar_mul(t0, t0, inv)
            nc.vector.tensor_mul(t1, mr, mr)
            nc.vector.tensor_sub(t0, t0, t1)
            nc.vector.tensor_mul(t1, mi, mi)
            nc.vector.tensor_add(t0, t0, t1)
            nc.vector.tensor_mul(t0, t0, xsc[r_, 1:2])        # descale var (1/s^2)
            nc.vector.tensor_scalar_add(t0, t0, EPS)          # t0 = zr
            # zi = 2*(eri/N - mr*mi)
            nc.vector.tensor_scalar_mul(t1, A(4), inv)
            nc.vector.tensor_mul(t2, mr, mi)
            nc.vector.tensor_sub(t1, t1, t2)
            nc.vector.tensor_mul(t1, t1, xsc[r_, 1:2])        # descale var (1/s^2)
            nc.vector.tensor_scalar_mul(t1, t1, 2.0)          # t1 = zi
            # mag = sqrt(zr^2+zi^2)
            nc.vector.tensor_mul(t2, t0, t0)
            nc.vector.tensor_mul(t3, t1, t1)
            nc.vector.tensor_add(t2, t2, t3)
            nc.scalar.sqrt(t2, t2)                            # t2 = mag
            # sr = sqrt((mag+zr)/2); si = zi/(2 sr)
            nc.vector.tensor_add(t3, t2, t0)
            nc.scalar.activation(sr, t3, mybir.ActivationFunctionType.Sqrt, scale=0.5)
            nc.vector.reciprocal(t3, sr)
            nc.vector.tensor_mul(si, t1, t3)
            nc.vector.tensor_scalar_mul(si, si, 0.5)          # si = zi/(2 sr)
            nc.vector.reciprocal(t3, t2)                      # t3 = 1/mag
            # fold 1/s into A so A' applies directly to the scaled q/v planes
            # (B then uses A' * scaled-mean = A * mean, exact)
            nc.vector.tensor_mul(t3, t3, xsc[r_, 0:1])
            # Ar = (qsr*sr + qsi*si)/mag ; Ai = (qsi*sr - qsr*si)/mag
            nc.vector.tensor_mul(t0, P(0), sr)
            nc.vector.tensor_mul(t1, P(1), si)
            nc.vector.tensor_add(t0, t0, t1)
            nc.vector.tensor_mul(C(0), t0, t3)
            nc.vector.tensor_mul(t0, P(1), sr)
            nc.vector.tensor_mul(t1, P(0), si)
            nc.vector.tensor_sub(t0, t0, t1)
            nc.vector.tensor_mul(C(1), t0, t3)
            # Br = qbr - Ar*mr + Ai*mi ; Bi = qbi - Ar*mi - Ai*mr
            nc.vector.tensor_mul(t0, C(0), mr)
            nc.vector.tensor_sub(t0, P(2), t0)
            nc.vector.tensor_mul(t1, C(1), mi)
            nc.vector.tensor_add(C(2), t0, t1)
            nc.vector.tensor_mul(t0, C(0), mi)
            nc.vector.tensor_sub(t0, P(3), t0)
            nc.vector.tensor_mul(t1, C(1), mr)
            nc.vector.tensor_sub(C(3), t0, t1)

        def bn_apply(planes, rows, coff):
            r_ = slice(0, rows)
            C = lambda c: coef[r_, coff + c:coff + c + 1]
            for b in range(BL):
                pr, pi = planes[b][0][r_, :], planes[b][1][r_, :]
                s0, s1 = scr16[0][r_, :], scr16[1][r_, :]
                nc.vector.tensor_scalar_mul(s1, pr, C(1))     # s1 = C1*re
                nc.vector.tensor_scalar(pr, pr, C(0), C(2),
                                        mybir.AluOpType.mult, mybir.AluOpType.add)
                nc.vector.tensor_scalar_mul(s0, pi, C(1))     # s0 = C1*im
                nc.vector.tensor_sub(pr, pr, s0)              # re' done
                nc.vector.tensor_scalar(pi, pi, C(0), C(3),
                                        mybir.AluOpType.mult, mybir.AluOpType.add)
                nc.vector.tensor_add(pi, pi, s1)              # im' done

        # v path first: it gates the lam_p matmuls
        bn_coef(32, 5, 4, 4)
        bn_apply(v16, 32, 4)

        qT = [[per.tile([128, 1024], F16, tag=f"qT{b}{ri}", name=f"qT{b}{ri}")
               for ri in range(2)] for b in range(BL)]
        V_rhs = per.tile([128, 1024], F16, tag="vrhs", name="vrhs")
        with tc.tile_pool(name="tq", bufs=2, space="PSUM") as tqp:
            for b in range(BL):
                for ri in range(2):
                    # V_rhs[(m),(ch,b,ri,v)] from v16: 8 transposes -> PV8, 1 copy
                    PV8 = tqp.tile([128, 256], F16, tag="PV8", name="PV8")
                    for ch in range(8):
                        nc.tensor.transpose(PV8[:, ch * 32:(ch + 1) * 32],
                                            v16[b][ri][:, ch * 128:(ch + 1) * 128],
                                            ident16[0:32, 0:32])
                    dstv = V_rhs[:, :].copy()
                    dstv.ap = bass_rust.VecI64Pair([(1024, 128), (128, 8), (1, 32)])
                    dstv.offset = b * 64 + ri * 32
                    nc.vector.tensor_copy(dstv, PV8[:])

            # lam_c
            lam_sb = per.tile([16, 128], F16, tag="lamc", name="lamc")
            for b in range(BL):
                plc = tqp.tile([16, 64], F32, tag="plc", name="plc")
                for ch in range(8):
                    rhs = V_rhs[:, :].copy()
                    rhs.ap = bass_rust.VecI64Pair([(1024, 128), (1, 64)])
                    rhs.offset = ch * 128 + b * 64
                    nc.tensor.matmul(plc[:], ksmT[b][:, ch * 16:(ch + 1) * 16], rhs,
                                     start=(ch == 0), stop=(ch == 7))
                nc.vector.tensor_copy(lam_sb[:, b * 64:(b + 1) * 64], plc[:])

            _tqp_keep = tqp

        # ---------------- main loop: nb-outer, k-inner ----------------
        # qds [128 (g,k), 1024 (g,h,t)]: block-diag q (zeros off-diag persist)
        qds = [per.tile([128, 4096], F16, tag=f"qds{p}", name=f"qds{p}")
               for p in range(2)]
        # full Y kept in SBUF (fp16, scaled by OSCALE) for the int8 output pass
        ypers = [[per.tile([64, 4096], F16, tag=f"yp{b}{ri}", name=f"yp{b}{ri}")
                  for ri in range(2)] for b in range(BL)]
        for p in range(2):
            nc.vector.memset(qds[p][:], 0.0)
            nc.sync.dma_start(qdram[p], qds[p][:])


        def _qds_build(nbq, parq, qdsp, qdpp):
            for bq in range(BL):
                for ri in range(2):
                    qkT = qdsp.tile([16, 1024], F16, tag="qkT", name="qkT")
                    PT8 = qdpp.tile([16, 1024], F16, tag="PT8", name="PT8")
                    for h in range(8):
                        nc.tensor.transpose(
                            PT8[:, h * 128:(h + 1) * 128],
                            qT[bq][ri][:, nbq * 128 + h * 16:
                                       nbq * 128 + h * 16 + 16],
                            ident16[:])
                    dst = qkT[:, :].copy()
                    dst.ap = bass_rust.VecI64Pair(
                        [(1024, 16), (128, 8), (16, 8), (1, 16)])   # (k),g,h,t
                    dst.offset = 0
                    srcp = PT8[:, :].copy()
                    srcp.ap = bass_rust.VecI64Pair(
                        [(1024, 16), (1, 8), (128, 8), (8, 16)])    # (k),g,h,t
                    srcp.offset = 0
                    nc.vector.tensor_copy(dst, srcp)
                    sapq = qkT[:, :].copy()
                    sapq.ap = bass_rust.VecI64Pair(
                        [(1024, 16), (128, 8), (1, 128)])    # (k, g, ht)
                    sapq.offset = 0
                    dapq = qdram[0][0:1, 0:1].copy()
                    dapq.ap = bass_rust.VecI64Pair(
                        [(4096, 16), (65664, 8), (1, 128)])  # (k, g, ht)
                    dapq.offset = parq * 524288 + (bq * 2 + ri) * 1024
                    nc.sync.dma_start(dapq, sapq)
            nc.scalar.dma_start(qds[parq][:], qdram[parq])

        with tc.tile_pool(name="lp", bufs=2, space="PSUM") as lpp, \
             tc.tile_pool(name="la", bufs=2) as lap, \
             tc.tile_pool(name="qdp", bufs=1, space="PSUM") as qdpp, \
             tc.tile_pool(name="qk", bufs=2) as qdsp, \
             tc.tile_pool(name="lyp", bufs=2) as lypp, \
             tc.tile_pool(name="yp", bufs=1, space="PSUM") as ypp:
            for nb in range(8):
                par = nb % 2
                if nb > 1:
                    _qds_build(nb, par, qdsp, qdpp)
                if nb > 0:
                    d = 7 - nb
                    nc.scalar.dma_start(
                        mkc[:, (d % 9) * 4096:(d % 9 + 1) * 4096], mk_d[d])
                lam_t = lap.tile([128, 2048], F16, tag="lam", name="lam")
                for kp in range(8):
                    P1P = lpp.tile([128, 256], F32, tag="P1P", name="P1P")
                    P2P = lpp.tile([128, 256], F32, tag="P2P", name="P2P")
                    for kk in range(2):
                        k = kp * 2 + kk
                        P1 = P1P[:, kk * 128:kk * 128 + 128]
                        P2 = P2P[:, kk * 128:kk * 128 + 128]
                        nc.tensor.matmul(P1, eye[:, k * 128:(k + 1) * 128], lam_sb[:],
                                         start=True, stop=False)
                        for bip in range(8):
                            d = (bip - nb + 7)
                            co = (d % 9) * 4096 + k * 256
                            rhs = V_rhs[:, bip * 128:(bip + 1) * 128]
                            nc.tensor.matmul(P1, mkc[:, co:co + 128], rhs,
                                             start=False, stop=(bip == 7))
                            nc.tensor.matmul(P2, mkc[:, co + 128:co + 256], rhs,
                                             start=(bip == 0), stop=(bip == 7))
                    # stage P2P in SBUF (single-PSUM-operand rule), then combine
                    p2s = lypp.tile([128, 256], F32, tag="p2s", name="p2s")
                    nc.scalar.copy(p2s[:], P2P[:])

                    def _ap3(t_, pitch, kstride, off):
                        a = t_[:, :].copy() if hasattr(t_, 'tag') else t_.copy()
                        a.ap = bass_rust.VecI64Pair(
                            [(pitch, 128), (kstride, 2), (64, 2), (1, 32)])
                        a.offset = off
                        return a
                    nc.vector.tensor_sub(_ap3(lam_t, 2048, 128, kp * 256),
                                         _ap3(P1P, 256, 128, 0),
                                         _ap3(p2s, 256, 128, 32))
                    nc.vector.tensor_add(_ap3(lam_t, 2048, 128, kp * 256 + 32),
                                         _ap3(P1P, 256, 128, 32),
                                         _ap3(p2s, 256, 128, 0))
                if nb == 0:
                    # q path: emitted after nb0's chains so it doesn't block PE
                    bn_coef(128, 0, 0, 0)
                    bn_apply(q16, 128, 0)
                    for bq in range(BL):
                        for ri in range(2):
                            for nbq in range(8):
                                pqz = qdpp.tile([128, 128], F16, tag="pqz", name="pqz")
                                nc.tensor.transpose(
                                    pqz[:],
                                    q16[bq][ri][:, nbq * 128:(nbq + 1) * 128],
                                    ident16[:])
                                nc.vector.tensor_copy(
                                    qT[bq][ri][:, nbq * 128:(nbq + 1) * 128], pqz[:])
                    _qds_build(0, 0, qdsp, qdpp)
                    _qds_build(1, 1, qdsp, qdpp)
                # lam roundtrip: two half stores (first overlaps second half's chains)
                nc.sync.dma_start(lamdram[par][:, 0:1024], lam_t[:, 0:1024])
                nc.sync.dma_start(lamdram[par][:, 1024:2048], lam_t[:, 1024:2048])
                lamyps = []
                for b in range(BL):
                    lamyp = lypp.tile([128, 1024], F16, tag=f"lamyp{b}", name=f"lamyp{b}")
                    lamyps.append(lamyp)
                    sap = lamdram[0][0:1, 0:1].copy()
                    sap.ap = bass_rust.VecI64Pair(
                        [(128, 128), (16384, 16), (1, 64)])   # ((g,k), t, riv)
                    sap.offset = par * 262144 + b * 64
                    dap = lamyp[:, :].copy()
                    dap.ap = bass_rust.VecI64Pair(
                        [(1024, 128), (64, 16), (1, 64)])     # ((g,k), t, riv)
                    dap.offset = 0
                    nc.scalar.dma_start(dap, sap)
                # Yp: per-t matmuls into [64,512] PSUM halves + strided combines
                # (combined values land directly in the persistent ypers tiles)
                def _apy(b_, ri_, off):
                    a = ypers[b_][ri_][:, :].copy()
                    a.ap = bass_rust.VecI64Pair([(4096, 64), (32, 8), (1, 32)])
                    a.offset = nb * 512 + off
                    return a

                def _apP(t_, off):
                    a = t_[:, :].copy()
                    a.ap = bass_rust.VecI64Pair([(512, 64), (64, 8), (1, 32)])
                    a.offset = off
                    return a
                for b in range(BL):
                    lamyp = lamyps[b]
                    for th in range(2):
                        P1Y = ypp.tile([64, 512], F32, tag="P1Y", name="P1Y")
                        P2Y = ypp.tile([64, 512], F32, tag="P2Y", name="P2Y")
                        for tt in range(8):
                            t = th * 8 + tt
                            for P, ri in ((P1Y, 0), (P2Y, 1)):
                                lhs = qds[par][:, :].copy()
                                lhs.ap = bass_rust.VecI64Pair(
                                    [(4096, 128), (128, 8), (16, 8)])  # part, g, h
                                lhs.offset = (b * 2 + ri) * 1024 + t
                                nc.tensor.matmul(P[:, tt * 64:(tt + 1) * 64], lhs,
                                                 lamyp[:, t * 64:(t + 1) * 64],
                                                 start=True, stop=True)
                        p2y = lypp.tile([64, 512], F32, tag="p2y", name="p2y")
                        nc.scalar.copy(p2y[:], P2Y[:])
                        oc = th * 256
                        nc.vector.tensor_sub(_apy(b, 0, oc),
                                             _apP(P1Y, 0), _apP(p2y, 32))
                        nc.vector.tensor_add(_apy(b, 1, oc),
                                             _apP(P1Y, 32), _apP(p2y, 0))
        # ---- int8 output quantization: per partition-row abs-max ----
        with tc.tile_pool(name="qz", bufs=2) as qzp:
            ysc_t = per.tile([64, 4], F32, tag="ysct", name="ysct")
            for b in range(BL):
                for ri in range(2):
                    mxt = qzp.tile([64, 1], F32, tag="mxt", name="mxt")
                    mnt = qzp.tile([64, 1], F32, tag="mnt", name="mnt")
                    nc.vector.tensor_reduce(mxt[:], ypers[b][ri][:],
                                            mybir.AxisListType.X,
                                            mybir.AluOpType.max)
                    nc.vector.tensor_reduce(mnt[:], ypers[b][ri][:],
                                            mybir.AxisListType.X,
                                            mybir.AluOpType.min)
                    nc.vector.tensor_scalar_mul(mnt[:], mnt[:], -1.0)
                    nc.vector.tensor_scalar_max(mxt[:], mxt[:], mnt[:])
                    nc.vector.tensor_scalar_add(mxt[:], mxt[:], 1e-12)
                    col = b * 2 + ri
                    # inv = absmax/127 (host multiplier); s = 1/inv
                    nc.vector.tensor_scalar_mul(ysc_t[:, col:col + 1], mxt[:],
                                                1.0 / 127.0)
                    st = qzp.tile([64, 1], F32, tag="stq", name="stq")
                    nc.vector.reciprocal(st[:], ysc_t[:, col:col + 1])
                    yqt = qzp.tile([64, 4096], I8, tag="yqt", name="yqt")
                    nc.vector.tensor_scalar_mul(yqt[:], ypers[b][ri][:], st[:])
                    nc.sync.dma_start(yq_d[b][ri], yqt[:])
            nc.sync.dma_start(ysc_d[:], ysc_t[:])

    nc.compile()
    return nc


def _get_state():
    if "state" in _CACHE:
        return _CACHE["state"]
    import jax
    from jax.sharding import Mesh, PartitionSpec, NamedSharding
    from jax.experimental.shard_map import shard_map
    from concourse import bass2jax as b2j
    import concourse.mybir as _mybir

    nc = _build_nc()
    b2j.install_neuronx_cc_hook()

    in_names, out_names, out_avals, in_meta = [], [], [], []
    for alloc in nc.m.functions[0].allocations:
        if not isinstance(alloc, _mybir.MemoryLocationSet):
            continue
        name = alloc.memorylocations[0].name
        if alloc.kind == "ExternalInput":
            in_names.append(name)
            in_meta.append((tuple(alloc.tensor_shape), _mybir.dt.np(alloc.dtype)))
        elif alloc.kind == "ExternalOutput":
            out_names.append(name)
            out_avals.append(jax.core.ShapedArray(
                tuple(alloc.tensor_shape), _mybir.dt.np(alloc.dtype)))

    partition_name = nc.partition_id_tensor.name if nc.partition_id_tensor else None
    if partition_name is not None:
        in_names.remove(partition_name)

    def _body(*args):
        operands = list(args)
        if partition_name is not None:
            operands.append(b2j.partition_id_tensor())
        outs = b2j._bass_exec_p.bind(
            *operands,
            out_avals=tuple(out_avals),
            in_names=tuple(in_names + ([partition_name] if partition_name else [])),
            out_names=tuple(out_names),
            lowering_input_output_aliases=(),
            sim_require_finite=True,
            sim_require_nnan=True,
            nc=nc)
        return tuple(outs)

    devs = jax.devices()[:NCORES]
    mesh = Mesh(np.asarray(devs), ("core",))
    P = PartitionSpec
    jf = jax.jit(shard_map(_body, mesh=mesh,
                           in_specs=(P("core"),) * len(in_names),
                           out_specs=(P("core"),) * len(out_names),
                           check_rep=False),
                 keep_unused=True)
    sh = NamedSharding(mesh, P("core"))
    try:
        # AOT-compile with bass_effect suppressed: C++ fast-path dispatch
        avals = [jax.ShapeDtypeStruct((NCORES * s[0],) + s[1:], d, sharding=sh)
                 for s, d in in_meta]
        fn = b2j.fast_dispatch_compile(lambda: jf.lower(*avals).compile())
    except Exception:
        fn = jf
    state = {"fn": fn, "sh": sh, "in_names": in_names, "out_names": out_names,
             "const_key": None, "const_dev": None}
    _CACHE["state"] = state
    return state


def _ensure_consts(state, inp):
    import jax
    key = tuple(inp[k].tobytes() for k in _CONST_KEYS)
    if state["const_key"] == key:
        return
    consts = _build_host_consts(inp)
    consts["xsc"] = np.ones((128, 2), np.float32)  # input scale s = 1
    dev = {}
    for name, arr in consts.items():
        rep = np.tile(arr, (NCORES,) + (1,) * (arr.ndim - 1))
        dev[name] = jax.device_put(rep, state["sh"])
    for v in dev.values():
        v.block_until_ready()
    state["const_dev"] = dev
    state["const_key"] = key


def _pool():
    if "pool" not in _CACHE:
        from concurrent.futures import ThreadPoolExecutor
        _CACHE["pool"] = ThreadPoolExecutor(4)
    return _CACHE["pool"]


def _build_xf(inp):
    # xf[core, bl*4 + ri*2 + cc] = x(ri)[core*2+bl, cc*128:(cc+1)*128, :] fp16
    xf = np.empty((NCORES, BL, 2, 2, 128, N2), np.float16)

    def _cvt(ri):
        src = inp['x_re'] if ri == 0 else inp['x_im']
        xf[:, :, ri] = src.reshape(NCORES, BL, 2, 128, N2)
    list(_pool().map(_cvt, (0, 1)))
    return xf.reshape(NCORES * 8, 128, N2)


def _assemble(yq, ysc):
    # yq: (B, 2, 64, 4096) int8; ysc: (NCORES*64, 4) inverse scales, col=b*2+ri
    # yc[b, g, h, nt, v] -> out[b, (h,v), (nt,g)], written straight into the
    # complex output's real/imag planes
    inv = ysc.reshape(NCORES, 64, BL, 2).transpose(0, 2, 3, 1).reshape(B, 2, 64, 1)
    inv = inv * np.float32(1.0 / _OSCALE)
    out = np.empty((B, 256, 1024), np.complex64)
    of = out.view(np.float32).reshape(B, 8, 32, 128, 8, 2)
    yqv = yq.reshape(B, 2, 8, 8, 128, 32)
    iv = inv.reshape(B, 2, 8, 8, 1, 1)
    of[..., 0] = (yqv[:, 0] * iv[:, 0]).transpose(0, 2, 4, 3, 1)
    of[..., 1] = (yqv[:, 1] * iv[:, 1]).transpose(0, 2, 4, 3, 1)
    return out.reshape(B, 256, 32, 32)


def kernel(**inputs):
    inp = {k: np.asarray(v) for k, v in inputs.items()}
    state = _get_state()
    _ensure_consts(state, inp)
    args = {"xf": _build_xf(inp), **state["const_dev"]}
    outs = state["fn"](*[args[n] for n in state["in_names"]])
    # fetch both outputs concurrently: the tiny ysc costs a full tunnel RTT,
    # which overlaps the bulk yq fetch this way
    futs = [_pool().submit(np.asarray, o) for o in outs]
    res = {n: futs[i].result() for i, n in enumerate(state["out_names"])}
    return _assemble(res["yq"].reshape(B, 2, 64, 4096), res["ysc"])


# revision 32
# speedup vs baseline: 1.1165x; 1.1165x over previous
# Trainium2 Bass kernel for nn_ComplexLambdaLayer (complex lambda attention layer).
# Sharding: data-parallel over batch b (16) across 8 cores (2 per core).
# The positional-lambda contraction lam_p[b,n,k,v] = sum_m R_k[n,m] V[b,v,m] uses
# the block-Toeplitz structure of R (R[n,m] = emb[pos_m - pos_n + 31]): only 15
# distinct 128x128 blocks per k exist (host-expanded fp16 table, d-major), so the
# 1024x1024 matmul becomes 8x8 chunk-matmuls with 15 stationary weights.
# lam_c is folded into the same PSUM chain via an indicator-row matmul.
# Yp = sum_k q*Lam uses a block-diagonal q lhsT (8 n-positions x 16 k = K128).
#
# Device schedule: nb-outer / k-inner main loop with a 9-slot rolling ring of mk
# d-blocks in SBUF; the BN AllReduce overlaps softmax/ksmT; the post-AR v-path is
# emitted first because it gates the matmuls.  TimelineSim ~291us.
#
# End-to-end wall time over the axon tunnel (~46 MB/s up, ~30 MB/s down, ~85 ms
# RTT) is dominated by host<->device transfer and per-call jit rebuild (the
# baseline re-jitted and re-shipped ~200MB per call, 3.7-4.4 s), so the runner:
#   - builds the Bass module and the jitted shard_map callable ONCE (_CACHE);
#   - keeps all weight-derived constant tables (mk 15.7MB/core, wstk, eyerow,
#     ident, bnp) device-resident across calls, revalidated by byte-compare of
#     the small weight inputs;
#   - ships x as fp16 (16MB/call up; coarser is impossible: the complex-BN
#     variance is a near-cancelling difference, amplifying input quantization
#     noise ~40x+, verified by numpy sim: int8 x -> 51% error);
#   - returns Y int8-quantized per partition row with f32 inverse scales
#     (8.4MB/call down, adds ~0.4% of row-max error), fetched concurrently so
#     the tiny scale tensor's RTT hides under the bulk fetch;
#   - uploads no donated zero output buffers (kernel writes every output byte).
# Steady-state call: ~0.6 s (upload 0.35 + download 0.25 + ~0.1 host/dispatch).
import numpy as np
from contextlib import ExitStack

import bass_rust
import concourse.bacc as bacc
import concourse.tile as tile
from concourse import mybir

F32 = mybir.dt.float32
F16 = mybir.dt.float16
I8 = mybir.dt.int8

NCORES = 8
B = 16
BL = 2          # batches per core
DIM = 256
KD = 16         # DIM_K
HEADS = 8
VD = 32         # DIM_V
N2 = 1024
EPS = 1e-5
NSTAT = float(B * N2)

_CACHE = {}

# Y is stored as Y*_OSCALE in fp16 on device (power of two: exact rescale).
_OSCALE = 1.0 / 16.0

_CONST_KEYS = ('wq_re', 'wq_im', 'wk_re', 'wk_im', 'wv_re', 'wv_im',
               'qs_re', 'qs_im', 'qb_re', 'qb_im', 'vs_re', 'vs_im',
               'vb_re', 'vb_im', 'emb_re', 'emb_im')


def _build_host_consts(inp):
    # --- M_all: lhsT[(m-chunk),(n-chunk)] = R[n,m] = emb[pos_m - pos_n + 31]
    # M[k, dp+7][ap*32+jp, a*32+j] = emb[4dp + ap - a + 31, jp - j + 31, k, 0]
    er, ei = inp['emb_re'], inp['emb_im']
    a = np.arange(4); j = np.arange(32); dp = np.arange(-7, 8)
    r0 = (4 * dp[:, None, None, None, None] + a[None, :, None, None, None]
          - a[None, None, None, :, None] + 31)
    r1 = j[None, None, :, None, None] - j[None, None, None, None, :] + 31
    r0 = np.broadcast_to(r0, (15, 4, 32, 4, 32))
    r1 = np.broadcast_to(r1, (15, 4, 32, 4, 32))
    Mr = np.moveaxis(er[r0, r1, :, 0], -1, 0).reshape(16, 15, 128, 128)
    Mi = np.moveaxis(ei[r0, r1, :, 0], -1, 0).reshape(16, 15, 128, 128)
    # mk layout: d-major [d 15][p 128][(k,ri,c) 4096] fp16 (rolling-ring loads)
    # Scaled by OSCALE so Y (which can reach ~1e5 and overflow fp16) is stored
    # as Y*OSCALE in the fp16 outputs; host assembly multiplies back.
    mk = np.empty((15, 128, 16 * 2 * 128), np.float16)
    for k in range(16):
        mk[:, :, k * 256:k * 256 + 128] = Mr[k].transpose(0, 1, 2)
        mk[:, :, k * 256 + 128:k * 256 + 256] = Mi[k]
    mk *= np.float16(_OSCALE)

    # --- W stacks: proj out_r chain uses [Wr; -Wi], out_i chain [Wi; Wr]
    # o-order: 0-127 q(h,k), 128-143 k-proj, 144-175 v-proj; lhsT layout [c,o]
    # q channel order h*16+k (h-major, so qTg k-col slices are contiguous)
    Wr = np.concatenate([inp['wq_re'], inp['wk_re'], inp['wv_re']], 0).T
    Wi = np.concatenate([inp['wq_im'], inp['wk_im'], inp['wv_im']], 0).T
    # cols: 0-127 q, 128-143 k, 144-159 pad, 160-191 v (32-aligned psum bases)
    Wr = np.concatenate([Wr[:, :144], np.zeros((256, 16), np.float32), Wr[:, 144:]], 1)
    Wi = np.concatenate([Wi[:, :144], np.zeros((256, 16), np.float32), Wi[:, 144:]], 1)
    wstk = np.empty((2, 2, 2, 128, 192), np.float16)  # [outri][cc][ri_in]
    for cc in range(2):
        sl = slice(cc * 128, (cc + 1) * 128)
        wstk[0, cc, 0] = Wr[sl]; wstk[0, cc, 1] = -Wi[sl]
        wstk[1, cc, 0] = Wi[sl]; wstk[1, cc, 1] = Wr[sl]

    # --- eyerow for lam_c fold: [16, 16*128] fp16, eyerow[kk, k*128+c] = (kk==k)
    # (scaled by OSCALE like mk so lam_c and lam_p carry the same factor)
    eyerow = np.zeros((16, 16 * 128), np.float16)
    for k in range(16):
        eyerow[k, k * 128:(k + 1) * 128] = np.float16(_OSCALE)

    ident = np.eye(128, dtype=np.float32)
    ident16 = np.eye(128, dtype=np.float16)

    # --- BN params tile [128, 8]: q Ar-src cols 0-3 (qs_r qs_i qb_r qb_i),
    # v on rows 0-31 cols 4-7
    bnp = np.zeros((128, 8), np.float32)
    bnp[:, 0] = inp['qs_re']; bnp[:, 1] = inp['qs_im']
    bnp[:, 2] = inp['qb_re']; bnp[:, 3] = inp['qb_im']
    bnp[:32, 4] = inp['vs_re']; bnp[:32, 5] = inp['vs_im']
    bnp[:32, 6] = inp['vb_re']; bnp[:32, 7] = inp['vb_im']
    return {"wstk": wstk.reshape(8, 128, 192), "mk": mk, "eyerow": eyerow,
            "ident": ident, "ident16": ident16, "bnp": bnp}


def _build_nc():
    nc = bacc.Bacc("TRN2", target_bir_lowering=False, num_devices=NCORES)
    # x arrives fp16. (int8 x was tried and fails: the complex-BN variance is a
    # near-cancelling difference, so the layer amplifies input quantization
    # noise ~30-100x through 1/sqrt(var+eps); fp16 is the coarsest viable.)
    # xsc (col0 = 1/s, col1 = 1/s^2) descales the BN variance and softmax |k|^2
    # for a host-chosen input scale s; with fp16 x it is fed s = 1.
    xf_d = nc.declare_dram_parameter("xf", [8, 128, N2], F16, isOutput=False)
    xsc_d = nc.declare_dram_parameter("xsc", [128, 2], F32, isOutput=False)
    w_d = nc.declare_dram_parameter("wstk", [8, 128, 192], F16, isOutput=False)
    mk_d = nc.declare_dram_parameter("mk", [15, 128, 4096], F16, isOutput=False)
    eye_d = nc.declare_dram_parameter("eyerow", [16, 2048], F16, isOutput=False)
    id_d = nc.declare_dram_parameter("ident", [128, 128], F32, isOutput=False)
    id16_d = nc.declare_dram_parameter("ident16", [128, 128], F16, isOutput=False)
    bnp_d = nc.declare_dram_parameter("bnp", [128, 8], F32, isOutput=False)
    # Y is returned int8-quantized per partition row (yq) with the inverse
    # scales in ysc[p, b*2+ri]; the host dequantizes and assembles.
    yq_d = nc.declare_dram_parameter("yq", [BL, 2, 64, 4096], I8, isOutput=True)
    ysc_d = nc.declare_dram_parameter("ysc", [64, 4], F32, isOutput=True)
    arin = nc.dram_tensor("arin", [128, 10], F32)
    arout = nc.dram_tensor("arout", [128, 10], F32, addr_space="Shared")
    lamdram = nc.dram_tensor("lamdram", [2, 128, 2048], F16)
    qdram = nc.dram_tensor("qdram", [2, 128, 4096], F16)

    with tile.TileContext(nc) as tc, ExitStack() as ctx:
        per = ctx.enter_context(tc.tile_pool(name="per", bufs=1))   # persistent
        tmp = ctx.enter_context(tc.tile_pool(name="tmp", bufs=2))   # scratch
        tmp1 = ctx.enter_context(tc.tile_pool(name="tmp1", bufs=1))  # scratch, single

        wt = [per.tile([128, 192], F16, tag=f"w{i}", name=f"w{i}") for i in range(8)]
        eye = per.tile([16, 2048], F16, tag="eye", name="eye")
        nc.sync.dma_start(eye[:], eye_d[:])
        ident = per.tile([128, 128], F32, tag="ident", name="ident")
        nc.sync.dma_start(ident[:], id_d[:])
        ident16 = per.tile([128, 128], F16, tag="ident16", name="ident16")
        nc.sync.dma_start(ident16[:], id16_d[:])
        bnp = per.tile([128, 8], F32, tag="bnp", name="bnp")
        nc.sync.dma_start(bnp[:], bnp_d[:])
        xsc = per.tile([128, 2], F32, tag="xsc", name="xsc")
        nc.sync.dma_start(xsc[:], xsc_d[:])

        # rolling 9-slot mk ring: slot s holds d-block with d % 9 == s
        mkc = per.tile([128, 9 * 4096], F16, tag="mkc", name="mkc")

        q16 = [[per.tile([128, N2], F16, tag=f"q16{b}{ri}", name=f"q16{b}{ri}")
                for ri in range(2)] for b in range(BL)]
        k_sb = [[per.tile([16, N2], F16, tag=f"k{b}{ri}", name=f"k{b}{ri}")
                 for ri in range(2)] for b in range(BL)]
        v16 = [[per.tile([32, N2], F16, tag=f"v16{b}{ri}", name=f"v16{b}{ri}")
                for ri in range(2)] for b in range(BL)]

        # ---------------- projections (fp16, N=512) ----------------
        with tc.tile_pool(name="xfp", bufs=1) as xfp, \
             tc.tile_pool(name="pj", bufs=4, space="PSUM") as pj:
            xft = [xfp.tile([128, N2], F16, tag=f"xf{i % 6}", name=f"xf{i}") for i in range(8)]
            for i in range(8):
                nc.sync.dma_start(wt[i][:], w_d[i])
            for i in range(6):
                nc.sync.dma_start(xft[i][:], xf_d[i])
            for i in range(6, 8):
                nc.sync.dma_start(xft[i][:], xf_d[i])
            for d in range(7, 15):
                # 1-elem dep copy: delays the big mkc load until all xf arrived
                nc.vector.tensor_copy(mkc[0:1, (d % 9) * 4096:(d % 9) * 4096 + 1],
                                      xft[7][0:1, 0:1])
                nc.scalar.dma_start(mkc[:, (d % 9) * 4096:(d % 9 + 1) * 4096], mk_d[d])
            for b in range(BL):
                for ri in range(2):
                    for nch in range(2):
                        pq = pj.tile([128, 512], F32, tag="pq", name="pq")
                        pkv = pj.tile([64, 512], F32, tag="pkv", name="pkv")
                        first = True
                        for cc in range(2):
                            for rin in range(2):
                                lhs = wt[ri * 4 + cc * 2 + rin]
                                rhs = xft[b * 4 + rin * 2 + cc][:, nch * 512:(nch + 1) * 512]
                                nc.tensor.matmul(pq[:], lhs[:, 0:128], rhs,
                                                 start=first, stop=(cc == 1 and rin == 1))
                                nc.tensor.matmul(pkv[:], lhs[:, 128:192], rhs,
                                                 start=first, stop=(cc == 1 and rin == 1))
                                first = False
                        sl = slice(nch * 512, (nch + 1) * 512)
                        nc.scalar.copy(q16[b][ri][:, sl], pq[:])
                        nc.scalar.copy(k_sb[b][ri][:, sl], pkv[0:16, :])
                        nc.scalar.copy(v16[b][ri][:, sl], pkv[32:64, :])

        # ---------------- BN stats + AllReduce ----------------
        stats = per.tile([128, 10], F32, tag="stats", name="stats")
        nc.vector.memset(stats[:], 0.0)
        st_sc = [tmp.tile([128, 1], F32, tag=f"sc{i}", name=f"sc{i}") for i in range(4)]
        scr16 = [tmp1.tile([128, N2], F16, tag=f"s16{i}", name=f"s16{i}") for i in range(2)]

        statsP = per.tile([128, 20], F32, tag="statsP", name="statsP")

        def stat5_b(planes, rows, coff, b):
            # one batch's 5 partial stats -> statsP[:, coff + b*5 + s]
            pr, pi = planes[b][0][0:rows, :], planes[b][1][0:rows, :]
            for s_i, expr in enumerate(["r", "i", "rr", "ii", "ri"]):
                t = statsP[0:rows, coff + b * 5 + s_i:coff + b * 5 + s_i + 1]
                if expr == "r":
                    nc.vector.tensor_reduce(t, pr, mybir.AxisListType.X,
                                            mybir.AluOpType.add)
                elif expr == "i":
                    nc.vector.tensor_reduce(t, pi, mybir.AxisListType.X,
                                            mybir.AluOpType.add)
                else:
                    a_, b_ = (pr, pr) if expr == "rr" else (pi, pi) if expr == "ii" else (pr, pi)
                    nc.vector.tensor_mul(scr16[b][0:rows, :], a_, b_)
                    nc.vector.tensor_reduce(t, scr16[b][0:rows, :],
                                            mybir.AxisListType.X, mybir.AluOpType.add)

        def stat5(planes, rows, coff):
            for s_i in range(5):
                nc.vector.tensor_add(
                    stats[0:rows, coff + s_i:coff + s_i + 1],
                    statsP[0:rows, coff * 2 + s_i:coff * 2 + s_i + 1],
                    statsP[0:rows, coff * 2 + 5 + s_i:coff * 2 + 5 + s_i + 1])

        stat5_b(q16, 128, 0, 0)
        stat5_b(v16, 32, 10, 0)
        stat5_b(q16, 128, 0, 1)
        stat5_b(v16, 32, 10, 1)
        stat5(q16, 128, 0)
        stat5(v16, 32, 5)
        nc.sync.dma_start(arin[:], stats[:])
        nc.gpsimd.collective_compute(
            "AllReduce", mybir.AluOpType.add,
            replica_groups=[list(range(NCORES))],
            ins=[arin[:]], outs=[arout[:]])
        ar = per.tile([128, 10], F32, tag="ar", name="ar")
        nc.sync.dma_start(ar[:], arout[:])

        # ---------------- softmax(|k|) -> ksmT (overlaps AllReduce) ----------------
        ksmT = [per.tile([128, 128], F16, tag=f"ksmT{b}", name=f"ksmT{b}") for b in range(BL)]
        scrap = [tmp1.tile([16, N2], F32, tag=f"scr{i}", name=f"scr{i}") for i in range(2)]
        with tc.tile_pool(name="tp", bufs=2, space="PSUM") as tpp:
            for b in range(BL):
                kr, ki = k_sb[b][0], k_sb[b][1]
                ka = scrap[0][0:16, :]
                t1 = scrap[1][0:16, :]
                nc.vector.tensor_mul(ka, kr, kr)
                nc.vector.tensor_mul(t1, ki, ki)
                nc.vector.tensor_add(ka, ka, t1)
                # |k|^2 carries the s^2 input scaling; descale before sqrt
                nc.vector.tensor_scalar_mul(ka, ka, xsc[0:16, 1:2])
                nc.scalar.sqrt(ka, ka)
                mx = st_sc[2][0:16, :]
                nc.vector.tensor_reduce(mx, ka, mybir.AxisListType.X, mybir.AluOpType.max)
                nc.vector.tensor_scalar(ka, ka, mx, None, mybir.AluOpType.subtract)
                sm = st_sc[3][0:16, :]
                nc.scalar.activation(ka, ka, mybir.ActivationFunctionType.Exp,
                                     accum_out=sm)
                rc = st_sc[2][0:16, :]
                nc.vector.reciprocal(rc, sm)
                nc.vector.tensor_scalar(ka, ka, rc, None, mybir.AluOpType.mult)
                for ch in range(8):
                    pt = tpp.tile([128, 16], F32, tag="pt", name="pt")
                    nc.tensor.transpose(pt[:], ka[:, ch * 128:(ch + 1) * 128],
                                        ident[0:16, 0:16])
                    nc.vector.tensor_copy(ksmT[b][:, ch * 16:(ch + 1) * 16], pt[:])

        # ---------------- BN coefficients ----------------
        coef = per.tile([128, 8], F32, tag="coef", name="coef")   # q: Ar Ai Br Bi cols0-3; v cols4-7
        ct = [tmp.tile([128, 1], F32, tag=f"ct{i}", name=f"ct{i}") for i in range(8)]

        def bn_coef(rows, soff, poff, coff):
            r_ = slice(0, rows)
            mr, mi, t0, t1, t2, t3, sr, si = (c[r_, :] for c in ct)
            A = lambda c: ar[r_, soff + c:soff + c + 1]
            P = lambda c: bnp[r_, poff + c:poff + c + 1]
            C = lambda c: coef[r_, coff + c:coff + c + 1]
            inv = 1.0 / NSTAT
            nc.vector.tensor_scalar_mul(mr, A(0), inv)
            nc.vector.tensor_scalar_mul(mi, A(1), inv)
            # zr = (err - eii)/N - mr^2 + mi^2 + EPS
            nc.vector.tensor_sub(t0, A(2), A(3))
            nc.vector.tensor_scalar_mul(t0, t0, inv)
            nc.vector.tensor_mul(t1, mr, mr)
            nc.vector.tensor_sub(t0, t0, t1)
            nc.vector.tensor_mul(t1, mi, mi)
            nc.vector.tensor_add(t0, t0, t1)
            nc.vector.tensor_mul(t0, t0, xsc[r_, 1:2])        # descale var (1/s^2)
            nc.vector.tensor_scalar_add(t0, t0, EPS)          # t0 = zr
            # zi = 2*(eri/N - mr*mi)
            nc.vector.tensor_scalar_mul(t1, A(4), inv)
            nc.vector.tensor_mul(t2, mr, mi)
            nc.vector.tensor_sub(t1, t1, t2)
            nc.vector.tensor_mul(t1, t1, xsc[r_, 1:2])        # descale var (1/s^2)
            nc.vector.tensor_scalar_mul(t1, t1, 2.0)          # t1 = zi
            # mag = sqrt(zr^2+zi^2)
            nc.vector.tensor_mul(t2, t0, t0)
            nc.vector.tensor_mul(t3, t1, t1)
            nc.vector.tensor_add(t2, t2, t3)
            nc.scalar.sqrt(t2, t2)                            # t2 = mag
            # sr = sqrt((mag+zr)/2); si = zi/(2 sr)
            nc.vector.tensor_add(t3, t2, t0)
            nc.scalar.activation(sr, t3, mybir.ActivationFunctionType.Sqrt, scale=0.5)
            nc.vector.reciprocal(t3, sr)
            nc.vector.tensor_mul(si, t1, t3)
            nc.vector.tensor_scalar_mul(si, si, 0.5)          # si = zi/(2 sr)
            nc.vector.reciprocal(t3, t2)                      # t3 = 1/mag
            # fold 1/s into A so A' applies directly to the scaled q/v planes
            # (B then uses A' * scaled-mean = A * mean, exact)
            nc.vector.tensor_mul(t3, t3, xsc[r_, 0:1])
            # Ar = (qsr*sr + qsi*si)/mag ; Ai = (qsi*sr - qsr*si)/mag
            nc.vector.tensor_mul(t0, P(0), sr)
            nc.vector.tensor_mul(t1, P(1), si)
            nc.vector.tensor_add(t0, t0, t1)
            nc.vector.tensor_mul(C(0), t0, t3)
            nc.vector.tensor_mul(t0, P(1), sr)
            nc.vector.tensor_mul(t1, P(0), si)
            nc.vector.tensor_sub(t0, t0, t1)
            nc.vector.tensor_mul(C(1), t0, t3)
            # Br = qbr - Ar*mr + Ai*mi ; Bi = qbi - Ar*mi - Ai*mr
            nc.vector.tensor_mul(t0, C(0), mr)
            nc.vector.tensor_sub(t0, P(2), t0)
            nc.vector.tensor_mul(t1, C(1), mi)
            nc.vector.tensor_add(C(2), t0, t1)
            nc.vector.tensor_mul(t0, C(0), mi)
            nc.vector.tensor_sub(t0, P(3), t0)
            nc.vector.tensor_mul(t1, C(1), mr)
            nc.vector.tensor_sub(C(3), t0, t1)

        def bn_apply(planes, rows, coff):
            r_ = slice(0, rows)
            C = lambda c: coef[r_, coff + c:coff + c + 1]
            for b in range(BL):
                pr, pi = planes[b][0][r_, :], planes[b][1][r_, :]
                s0, s1 = scr16[0][r_, :], scr16[1][r_, :]
                nc.vector.tensor_scalar_mul(s1, pr, C(1))     # s1 = C1*re
                nc.vector.tensor_scalar(pr, pr, C(0), C(2),
                                        mybir.AluOpType.mult, mybir.AluOpType.add)
                nc.vector.tensor_scalar_mul(s0, pi, C(1))     # s0 = C1*im
                nc.vector.tensor_sub(pr, pr, s0)              # re' done
                nc.vector.tensor_scalar(pi, pi, C(0), C(3),
                                        mybir.AluOpType.mult, mybir.AluOpType.add)
                nc.vector.tensor_add(pi, pi, s1)              # im' done

        # v path first: it gates the lam_p matmuls
        bn_coef(32, 5, 4, 4)
        bn_apply(v16, 32, 4)

        qT = [[per.tile([128, 1024], F16, tag=f"qT{b}{ri}", name=f"qT{b}{ri}")
               for ri in range(2)] for b in range(BL)]
        V_rhs = per.tile([128, 1024], F16, tag="vrhs", name="vrhs")
        with tc.tile_pool(name="tq", bufs=2, space="PSUM") as tqp:
            for b in range(BL):
                for ri in range(2):
                    # V_rhs[(m),(ch,b,ri,v)] from v16: 8 transposes -> PV8, 1 copy
                    PV8 = tqp.tile([128, 256], F16, tag="PV8", name="PV8")
                    for ch in range(8):
                        nc.tensor.transpose(PV8[:, ch * 32:(ch + 1) * 32],
                                            v16[b][ri][:, ch * 128:(ch + 1) * 128],
                                            ident16[0:32, 0:32])
                    dstv = V_rhs[:, :].copy()
                    dstv.ap = bass_rust.VecI64Pair([(1024, 128), (128, 8), (1, 32)])
                    dstv.offset = b * 64 + ri * 32
                    nc.vector.tensor_copy(dstv, PV8[:])

            # lam_c
            lam_sb = per.tile([16, 128], F16, tag="lamc", name="lamc")
            for b in range(BL):
                plc = tqp.tile([16, 64], F32, tag="plc", name="plc")
                for ch in range(8):
                    rhs = V_rhs[:, :].copy()
                    rhs.ap = bass_rust.VecI64Pair([(1024, 128), (1, 64)])
                    rhs.offset = ch * 128 + b * 64
                    nc.tensor.matmul(plc[:], ksmT[b][:, ch * 16:(ch + 1) * 16], rhs,
                                     start=(ch == 0), stop=(ch == 7))
                nc.vector.tensor_copy(lam_sb[:, b * 64:(b + 1) * 64], plc[:])

            _tqp_keep = tqp

        # ---------------- main loop: nb-outer, k-inner ----------------
        # qds [128 (g,k), 1024 (g,h,t)]: block-diag q (zeros off-diag persist)
        qds = [per.tile([128, 4096], F16, tag=f"qds{p}", name=f"qds{p}")
               for p in range(2)]
        # full Y kept in SBUF (fp16, scaled by OSCALE) for the int8 output pass
        ypers = [[per.tile([64, 4096], F16, tag=f"yp{b}{ri}", name=f"yp{b}{ri}")
                  for ri in range(2)] for b in range(BL)]
        for p in range(2):
            nc.vector.memset(qds[p][:], 0.0)
            nc.sync.dma_start(qdram[p], qds[p][:])


        def _qds_build(nbq, parq, qdsp, qdpp):
            for bq in range(BL):
                for ri in range(2):
                    qkT = qdsp.tile([16, 1024], F16, tag="qkT", name="qkT")
                    PT8 = qdpp.tile([16, 1024], F16, tag="PT8", name="PT8")
                    for h in range(8):
                        nc.tensor.transpose(
                            PT8[:, h * 128:(h + 1) * 128],
                            qT[bq][ri][:, nbq * 128 + h * 16:
                                       nbq * 128 + h * 16 + 16],
                            ident16[:])
                    dst = qkT[:, :].copy()
                    dst.ap = bass_rust.VecI64Pair(
                        [(1024, 16), (128, 8), (16, 8), (1, 16)])   # (k),g,h,t
                    dst.offset = 0
                    srcp = PT8[:, :].copy()
                    srcp.ap = bass_rust.VecI64Pair(
                        [(1024, 16), (1, 8), (128, 8), (8, 16)])    # (k),g,h,t
                    srcp.offset = 0
                    nc.vector.tensor_copy(dst, srcp)
                    sapq = qkT[:, :].copy()
                    sapq.ap = bass_rust.VecI64Pair(
                        [(1024, 16), (128, 8), (1, 128)])    # (k, g, ht)
                    sapq.offset = 0
                    dapq = qdram[0][0:1, 0:1].copy()
                    dapq.ap = bass_rust.VecI64Pair(
                        [(4096, 16), (65664, 8), (1, 128)])  # (k, g, ht)
                    dapq.offset = parq * 524288 + (bq * 2 + ri) * 1024
                    nc.sync.dma_start(dapq, sapq)
            nc.scalar.dma_start(qds[parq][:], qdram[parq])

        with tc.tile_pool(name="lp", bufs=2, space="PSUM") as lpp, \
             tc.tile_pool(name="la", bufs=2) as lap, \
             tc.tile_pool(name="qdp", bufs=1, space="PSUM") as qdpp, \
             tc.tile_pool(name="qk", bufs=2) as qdsp, \
             tc.tile_pool(name="lyp", bufs=2) as lypp, \
             tc.tile_pool(name="yp", bufs=1, space="PSUM") as ypp:
            for nb in range(8):
                par = nb % 2
                if nb > 1:
                    _qds_build(nb, par, qdsp, qdpp)
                if nb > 0:
                    d = 7 - nb
                    nc.scalar.dma_start(
                        mkc[:, (d % 9) * 4096:(d % 9 + 1) * 4096], mk_d[d])
                lam_t = lap.tile([128, 2048], F16, tag="lam", name="lam")
                for kp in range(8):
                    P1P = lpp.tile([128, 256], F32, tag="P1P", name="P1P")
                    P2P = lpp.tile([128, 256], F32, tag="P2P", name="P2P")
                    for kk in range(2):
                        k = kp * 2 + kk
                        P1 = P1P[:, kk * 128:kk * 128 + 128]
                        P2 = P2P[:, kk * 128:kk * 128 + 128]
                        nc.tensor.matmul(P1, eye[:, k * 128:(k + 1) * 128], lam_sb[:],
                                         start=True, stop=False)
                        for bip in range(8):
                            d = (bip - nb + 7)
                            co = (d % 9) * 4096 + k * 256
                            rhs = V_rhs[:, bip * 128:(bip + 1) * 128]
                            nc.tensor.matmul(P1, mkc[:, co:co + 128], rhs,
                                             start=False, stop=(bip == 7))
                            nc.tensor.matmul(P2, mkc[:, co + 128:co + 256], rhs,
                                             start=(bip == 0), stop=(bip == 7))
                    # stage P2P in SBUF (single-PSUM-operand rule), then combine
                    p2s = lypp.tile([128, 256], F32, tag="p2s", name="p2s")
                    nc.scalar.copy(p2s[:], P2P[:])

                    def _ap3(t_, pitch, kstride, off):
                        a = t_[:, :].copy() if hasattr(t_, 'tag') else t_.copy()
                        a.ap = bass_rust.VecI64Pair(
                            [(pitch, 128), (kstride, 2), (64, 2), (1, 32)])
                        a.offset = off
                        return a
                    nc.vector.tensor_sub(_ap3(lam_t, 2048, 128, kp * 256),
                                         _ap3(P1P, 256, 128, 0),
                                         _ap3(p2s, 256, 128, 32))
                    nc.vector.tensor_add(_ap3(lam_t, 2048, 128, kp * 256 + 32),
                                         _ap3(P1P, 256, 128, 32),
                                         _ap3(p2s, 256, 128, 0))
                if nb == 0:
                    # q path: emitted after nb0's chains so it doesn't block PE
                    bn_coef(128, 0, 0, 0)
                    bn_apply(q16, 128, 0)
                    for bq in range(BL):
                        for ri in range(2):
                            for nbq in range(8):
                                pqz = qdpp.tile([128, 128], F16, tag="pqz", name="pqz")
                                nc.tensor.transpose(
                                    pqz[:],
                                    q16[bq][ri][:, nbq * 128:(nbq + 1) * 128],
                                    ident16[:])
                                nc.vector.tensor_copy(
                                    qT[bq][ri][:, nbq * 128:(nbq + 1) * 128], pqz[:])
                    _qds_build(0, 0, qdsp, qdpp)
                    _qds_build(1, 1, qdsp, qdpp)
                # lam roundtrip: two half stores (first overlaps second half's chains)
                nc.sync.dma_start(lamdram[par][:, 0:1024], lam_t[:, 0:1024])
                nc.sync.dma_start(lamdram[par][:, 1024:2048], lam_t[:, 1024:2048])
                lamyps = []
                for b in range(BL):
                    lamyp = lypp.tile([128, 1024], F16, tag=f"lamyp{b}", name=f"lamyp{b}")
                    lamyps.append(lamyp)
                    sap = lamdram[0][0:1, 0:1].copy()
                    sap.ap = bass_rust.VecI64Pair(
                        [(128, 128), (16384, 16), (1, 64)])   # ((g,k), t, riv)
                    sap.offset = par * 262144 + b * 64
                    dap = lamyp[:, :].copy()
                    dap.ap = bass_rust.VecI64Pair(
                        [(1024, 128), (64, 16), (1, 64)])     # ((g,k), t, riv)
                    dap.offset = 0
                    nc.scalar.dma_start(dap, sap)
                # Yp: per-t matmuls into [64,512] PSUM halves + strided combines
                # (combined values land directly in the persistent ypers tiles)
                def _apy(b_, ri_, off):
                    a = ypers[b_][ri_][:, :].copy()
                    a.ap = bass_rust.VecI64Pair([(4096, 64), (32, 8), (1, 32)])
                    a.offset = nb * 512 + off
                    return a

                def _apP(t_, off):
                    a = t_[:, :].copy()
                    a.ap = bass_rust.VecI64Pair([(512, 64), (64, 8), (1, 32)])
                    a.offset = off
                    return a
                for b in range(BL):
                    lamyp = lamyps[b]
                    for th in range(2):
                        P1Y = ypp.tile([64, 512], F32, tag="P1Y", name="P1Y")
                        P2Y = ypp.tile([64, 512], F32, tag="P2Y", name="P2Y")
                        for tt in range(8):
                            t = th * 8 + tt
                            for P, ri in ((P1Y, 0), (P2Y, 1)):
                                lhs = qds[par][:, :].copy()
                                lhs.ap = bass_rust.VecI64Pair(
                                    [(4096, 128), (128, 8), (16, 8)])  # part, g, h
                                lhs.offset = (b * 2 + ri) * 1024 + t
                                nc.tensor.matmul(P[:, tt * 64:(tt + 1) * 64], lhs,
                                                 lamyp[:, t * 64:(t + 1) * 64],
                                                 start=True, stop=True)
                        p2y = lypp.tile([64, 512], F32, tag="p2y", name="p2y")
                        nc.scalar.copy(p2y[:], P2Y[:])
                        oc = th * 256
                        nc.vector.tensor_sub(_apy(b, 0, oc),
                                             _apP(P1Y, 0), _apP(p2y, 32))
                        nc.vector.tensor_add(_apy(b, 1, oc),
                                             _apP(P1Y, 32), _apP(p2y, 0))
        # ---- int8 output quantization: per partition-row abs-max ----
        with tc.tile_pool(name="qz", bufs=2) as qzp:
            ysc_t = per.tile([64, 4], F32, tag="ysct", name="ysct")
            for b in range(BL):
                for ri in range(2):
                    mxt = qzp.tile([64, 1], F32, tag="mxt", name="mxt")
                    mnt = qzp.tile([64, 1], F32, tag="mnt", name="mnt")
                    nc.vector.tensor_reduce(mxt[:], ypers[b][ri][:],
                                            mybir.AxisListType.X,
                                            mybir.AluOpType.max)
                    nc.vector.tensor_reduce(mnt[:], ypers[b][ri][:],
                                            mybir.AxisListType.X,
                                            mybir.AluOpType.min)
                    nc.vector.tensor_scalar_mul(mnt[:], mnt[:], -1.0)
                    nc.vector.tensor_scalar_max(mxt[:], mxt[:], mnt[:])
                    nc.vector.tensor_scalar_add(mxt[:], mxt[:], 1e-12)
                    col = b * 2 + ri
                    # inv = absmax/127 (host multiplier); s = 1/inv
                    nc.vector.tensor_scalar_mul(ysc_t[:, col:col + 1], mxt[:],
                                                1.0 / 127.0)
                    st = qzp.tile([64, 1], F32, tag="stq", name="stq")
                    nc.vector.reciprocal(st[:], ysc_t[:, col:col + 1])
                    yqt = qzp.tile([64, 4096], I8, tag="yqt", name="yqt")
                    nc.vector.tensor_scalar_mul(yqt[:], ypers[b][ri][:], st[:])
                    nc.sync.dma_start(yq_d[b][ri], yqt[:])
            nc.sync.dma_start(ysc_d[:], ysc_t[:])

    nc.compile()
    return nc


def _get_state():
    if "state" in _CACHE:
        return _CACHE["state"]
    import jax
    from jax.sharding import Mesh, PartitionSpec, NamedSharding
    from jax.experimental.shard_map import shard_map
    from concourse import bass2jax as b2j
    import concourse.mybir as _mybir

    nc = _build_nc()
    b2j.install_neuronx_cc_hook()

    in_names, out_names, out_avals, in_meta = [], [], [], []
    for alloc in nc.m.functions[0].allocations:
        if not isinstance(alloc, _mybir.MemoryLocationSet):
            continue
        name = alloc.memorylocations[0].name
        if alloc.kind == "ExternalInput":
            in_names.append(name)
            in_meta.append((tuple(alloc.tensor_shape), _mybir.dt.np(alloc.dtype)))
        elif alloc.kind == "ExternalOutput":
            out_names.append(name)
            out_avals.append(jax.core.ShapedArray(
                tuple(alloc.tensor_shape), _mybir.dt.np(alloc.dtype)))

    partition_name = nc.partition_id_tensor.name if nc.partition_id_tensor else None
    if partition_name is not None:
        in_names.remove(partition_name)

    def _body(*args):
        operands = list(args)
        if partition_name is not None:
            operands.append(b2j.partition_id_tensor())
        outs = b2j._bass_exec_p.bind(
            *operands,
            out_avals=tuple(out_avals),
            in_names=tuple(in_names + ([partition_name] if partition_name else [])),
            out_names=tuple(out_names),
            lowering_input_output_aliases=(),
            sim_require_finite=True,
            sim_require_nnan=True,
            nc=nc)
        return tuple(outs)

    devs = jax.devices()[:NCORES]
    mesh = Mesh(np.asarray(devs), ("core",))
    P = PartitionSpec
    jf = jax.jit(shard_map(_body, mesh=mesh,
                           in_specs=(P("core"),) * len(in_names),
                           out_specs=(P("core"),) * len(out_names),
                           check_rep=False),
                 keep_unused=True)
    sh = NamedSharding(mesh, P("core"))
    try:
        # AOT-compile with bass_effect suppressed: C++ fast-path dispatch
        avals = [jax.ShapeDtypeStruct((NCORES * s[0],) + s[1:], d, sharding=sh)
                 for s, d in in_meta]
        fn = b2j.fast_dispatch_compile(lambda: jf.lower(*avals).compile())
    except Exception:
        fn = jf
    state = {"fn": fn, "sh": sh, "in_names": in_names, "out_names": out_names,
             "const_key": None, "const_dev": None}
    _CACHE["state"] = state
    return state


def _ensure_consts(state, inp):
    import jax
    key = tuple(inp[k].tobytes() for k in _CONST_KEYS)
    if state["const_key"] == key:
        return
    consts = _build_host_consts(inp)
    consts["xsc"] = np.ones((128, 2), np.float32)  # input scale s = 1
    dev = {}
    for name, arr in consts.items():
        rep = np.tile(arr, (NCORES,) + (1,) * (arr.ndim - 1))
        dev[name] = jax.device_put(rep, state["sh"])
    for v in dev.values():
        v.block_until_ready()
    state["const_dev"] = dev
    state["const_key"] = key


def _pool():
    if "pool" not in _CACHE:
        from concurrent.futures import ThreadPoolExecutor
        _CACHE["pool"] = ThreadPoolExecutor(4)
    return _CACHE["pool"]


def _build_xf(inp):
    # xf[core, bl*4 + ri*2 + cc] = x(ri)[core*2+bl, cc*128:(cc+1)*128, :] fp16
    xf = np.empty((NCORES, BL, 2, 2, 128, N2), np.float16)

    def _cvt(ri):
        src = inp['x_re'] if ri == 0 else inp['x_im']
        xf[:, :, ri] = src.reshape(NCORES, BL, 2, 128, N2)
    list(_pool().map(_cvt, (0, 1)))
    return xf.reshape(NCORES * 8, 128, N2)


def _assemble(yq, ysc):
    # yq: (B, 2, 64, 4096) int8; ysc: (NCORES*64, 4) inverse scales, col=b*2+ri
    # yc[b, g, h, nt, v] -> out[b, (h,v), (nt,g)], written straight into the
    # complex output's real/imag planes
    inv = ysc.reshape(NCORES, 64, BL, 2).transpose(0, 2, 3, 1).reshape(B, 2, 64, 1)
    inv = inv * np.float32(1.0 / _OSCALE)
    out = np.empty((B, 256, 1024), np.complex64)
    of = out.view(np.float32).reshape(B, 8, 32, 128, 8, 2)
    yqv = yq.reshape(B, 2, 8, 8, 128, 32)
    iv = inv.reshape(B, 2, 8, 8, 1, 1)
    of[..., 0] = (yqv[:, 0] * iv[:, 0]).transpose(0, 2, 4, 3, 1)
    of[..., 1] = (yqv[:, 1] * iv[:, 1]).transpose(0, 2, 4, 3, 1)
    return out.reshape(B, 256, 32, 32)


def kernel(**inputs):
    inp = {k: np.asarray(v) for k, v in inputs.items()}
    state = _get_state()
    _ensure_consts(state, inp)
    args = {"xf": _build_xf(inp), **state["const_dev"]}
    outs = state["fn"](*[args[n] for n in state["in_names"]])
    # fetch both outputs concurrently: the tiny ysc costs a full tunnel RTT,
    # which overlaps the bulk yq fetch this way
    futs = [_pool().submit(np.asarray, o) for o in outs]
    res = {n: futs[i].result() for i, n in enumerate(state["out_names"])}
    return _assemble(res["yq"].reshape(B, 2, 64, 4096), res["ysc"])
